# revision 5
# baseline (speedup 1.0000x reference)
"""Causal multi-head self-attention on 8 Trainium2 NeuronCores.

Problem: x[4,2048,1024], Wq/Wk/Wv/Wo[1024,1024], H=16 heads, dk=64.
  q = x@Wq.T, k = x@Wk.T, v = x@Wv.T  (per-head causal softmax(q k^T/8) v) @ Wo.T

Sharding: core c handles batch b=c//2 and head-half hh=c%2 (8 heads).
Each core returns a partial output (its 512 attn columns through the
matching 512 rows of Wo.T); the host sums core pairs.

Precision plan (gate is 2e-2 max-err/max-ref):
  - Projections run as fp8e4m3 DoubleRow matmuls (contraction 256 per
    instruction, 0.5 cycles/row): x ships as a quantized pair
    X1=fp8(x), X2=fp8(x-X1) and the weights as W1=fp8(s*W),
    W2=fp8(s*W-W1); the three cross terms (dropping X2@W2) make the
    projection bf16-accurate at 75% of the bf16 matmul cost.  The
    scale s (64 for Wq/Wk, 16 for Wv) keeps the weights out of the
    fp8 denormal range; 64*64 folds into the exp scale and the 16
    into the ones-column of V.
  - Scores q k^T run in bf16 (no DoubleRow packing possible at dk=64).
  - PV runs TRANSPOSED: out[q,dk] tiles accumulate lhsT=probsT blocks
    x rhs=v blocks, so each matmul costs only 65 output columns
    instead of the q-width.  The ones-column of V accumulates the
    softmax denominator into column 64.  With PV_FP8 the probs are
    written as fp8e4m3 by the exp itself and a k-block PAIR contracts
    per DoubleRow matmul with fp8 V1/V2 (residual pair).
  - Normalization is per-partition on DVE (reciprocal + 4
    tensor_scalar muls per head) -- no PE broadcast needed in the
    transposed layout.
  - attn tiles are transposed back on PE (bf16, via identity) to feed
    the bf16 O-projection from the attnT layout.

The per-chunk pipeline interleaves the next chunk's projections and
the previous chunk's transposes + O-projection into the ACT-bound
attention phase as PE "fillers"; exp covers two k-blocks per
instruction; PV trails two steps behind the score matmuls.  Startup
warms the ACT exp table and the PE clock gate under the input DMAs.
"""

import numpy as np

import concourse.bass as bass
import concourse.masks as masks
import concourse.mybir as mybir
import concourse.tile as tile
from concourse.bass_utils import run_bass_kernel_spmd
from concourse.vector_clock import ScopedClock, VectorClock

B, S, D, H, DK = 4, 2048, 1024, 16, 64
HPC = H // 2          # heads per core
HD = HPC * DK         # 512 head-dim columns per core
CH = 512              # q-chunk width
NCH = S // CH         # 4
NKB = S // 128        # 16 k-blocks
F32 = mybir.dt.float32
F32R = mybir.dt.float32r
F8 = mybir.dt.float8e4
BF = mybir.dt.bfloat16
EXP = mybir.ActivationFunctionType.Exp
DRM = mybir.MatmulPerfMode.DoubleRow

XRES = True           # second fp8 pass for the x residual in projections
WRES = True           # second fp8 pass for the W residual in projections
PV_FP8 = True         # fp8 DoubleRow PV (faster, rel-err ~1.6e-2 vs ~4e-3)
WSC_QK = 64.0         # host-side scale on Wq/Wk (fp8 denormal avoidance)
WSC_V = 16.0          # host-side scale on Wv (bounded so 16*v fits e4m3)
EXP_SCALE = 0.125 / (WSC_QK * WSC_QK)  # = 2^-15 exactly
NXP = 2 if XRES else 1
NWP = 2 if WRES else 1
NPP = NXP + NWP - 1   # projection passes: X1W1(, X2W1)(, X1W2)
EDT = F8 if PV_FP8 else BF


def _drain_and_barrier_split(self, tick_clock, wait_clock):
    # The stock Tile tail drain attaches every outstanding sem wait to one
    # Drain instruction; this walrus build caps sync waits per instruction
    # and rejects it.  Put each wait on its own SP nop first, then drain
    # with no waits (SP has observed everything by then).
    gc = tick_clock.global_clock
    n = len(gc)
    for proc in range(n):
        t = gc[proc]
        if t == 0:
            continue
        vc = VectorClock([0] * n)
        vc.require_at_least(proc, t)
        nop = self.nc.sync.nop(nofuse=True)
        wait_clock.add_sem_waits(nop.ins, ScopedClock({None: vc}))
    self.nc.sync.drain()
    self.nc.all_engine_barrier()
    assert self.sems is not None
    popped = self.nc._tile_sem_poison_stack.pop()
    assert popped is self._sem_poison
    self.nc.clear_and_free_semaphores(list(self.sems.allocated().values()))
    self.nc.all_engine_barrier()


def _build_kernel(ctx, tc, xTs, wqTs, wkTs, wvTs, woT, out):
    nc = tc.nc
    KC = D // 128  # 8 contraction chunks of the projections

    wpool = ctx.enter_context(tc.tile_pool(name="weights", bufs=1))
    kvpool = ctx.enter_context(tc.tile_pool(name="kv", bufs=1))
    xpool = ctx.enter_context(tc.tile_pool(name="x", bufs=1))
    qpool = ctx.enter_context(tc.tile_pool(name="q", bufs=2))
    epool = ctx.enter_context(tc.tile_pool(name="exp", bufs=4))
    apool = ctx.enter_context(tc.tile_pool(name="attn", bufs=2))
    opool = ctx.enter_context(tc.tile_pool(name="osb", bufs=4))
    rpool = ctx.enter_context(tc.tile_pool(name="recip", bufs=2))
    # One PSUM pool, 8 banks: sc 2x[128,1024]f32 (4) + at 2x[128,4,128]f32
    # (2) + fill 1x[128,512]f32 (1) + tp 1x[128,4,128]bf16 (1).
    # Projection/O-proj groups share the "sc"/"fill" slots.
    pp = ctx.enter_context(tc.tile_pool(name="pp", bufs=2, space="PSUM"))

    # --- whole-kernel-resident tiles ---
    wq = [wpool.tile([128, KC, HD], F8, tag=f"wq{i}", name=f"wq{i}")
          for i in range(NWP)]
    wk = [wpool.tile([128, KC, HD], F8, tag=f"wk{i}", name=f"wk{i}")
          for i in range(NWP)]
    wv = [wpool.tile([128, KC, HD], F8, tag=f"wv{i}", name=f"wv{i}")
          for i in range(NWP)]
    wo = wpool.tile([128, 4, D], BF, tag="wo")
    ident = wpool.tile([128, 128], BF, tag="ident")
    kT = kvpool.tile([128, 4, S], BF, tag="kT")
    vdt = F8 if PV_FP8 else BF
    v1 = kvpool.tile([128, NKB, HPC, DK + 1], vdt, tag="v1")
    vs = [v1]
    if PV_FP8:
        v2 = kvpool.tile([128, NKB, HPC, DK + 1], F8, tag="v2")
        vs.append(v2)

    def dma_x(j):
        cs = slice(j * CH, (j + 1) * CH)
        xch = [xpool.tile([128, KC, CH], F8, tag=f"x{i}", name=f"x{i}")
               for i in range(NXP)]
        for i in range(NXP):
            nc.sync.dma_start(
                out=xch[i],
                in_=xTs[i].rearrange("(c p) n -> p c n", p=128)[:, :, cs])
        return xch

    # First matmul needs only xch0 + wq: interleave those DMAs first.
    cs0 = slice(0, CH)
    xch0 = [xpool.tile([128, KC, CH], F8, tag=f"x{i}", name=f"x{i}")
            for i in range(NXP)]
    for i in range(NWP):
        nc.gpsimd.dma_start(
            out=wq[i], in_=wqTs[i].rearrange("(c p) n -> p c n", p=128))
    for i in range(NXP):
        xe = nc.sync if i == 0 else nc.scalar
        xe.dma_start(
            out=xch0[i],
            in_=xTs[i].rearrange("(c p) n -> p c n", p=128)[:, :, cs0])
    for i in range(NWP):
        nc.gpsimd.dma_start(
            out=wk[i], in_=wkTs[i].rearrange("(c p) n -> p c n", p=128))
        nc.gpsimd.dma_start(
            out=wv[i], in_=wvTs[i].rearrange("(c p) n -> p c n", p=128))
    nc.sync.dma_start(out=wo, in_=woT.rearrange("(c p) n -> p c n", p=128))
    masks.make_identity(nc, ident[:, :])
    ones_f32 = wpool.tile([1, DK], F32, tag="ones_f32")
    nc.vector.memset(ones_f32, 1.0)
    vcol_f32 = wpool.tile([128, NKB, HPC, 1], F32, tag="vcol_f32")
    nc.vector.memset(vcol_f32, WSC_V)
    nc.vector.tensor_copy(v1[:, :, :, DK:DK + 1], vcol_f32)
    if PV_FP8:
        vcol0_f32 = wpool.tile([128, NKB, HPC, 1], F32, tag="vcol0_f32")
        nc.vector.memset(vcol0_f32, 0.0)
        nc.vector.tensor_copy(v2[:, :, :, DK:DK + 1], vcol0_f32)
    warm = wpool.tile([128, 128], F32R, tag="warm")
    warm_f32 = wpool.tile([128, 128], F32, tag="warm_f32")
    nc.vector.memset(warm_f32, 0.0)
    nc.vector.tensor_copy(warm, warm_f32)
    # preload the ACT exp table set under the input DMAs (~2.7us on HW)
    rcw = rpool.tile([1, DK], F32, tag="rcw", name="rcw", bufs=1)
    nc.scalar.activation(out=rcw, in_=ones_f32, func=EXP, scale=1.0)
    # hold the PE clock-gate open / absorb the cold ramp while DMAs land
    wps = pp.tile([128, 2 * CH], F32, tag="sc", name="wps")
    for r in range(12):
        nc.tensor.matmul(wps[:, (r % 2) * CH:(r % 2) * CH + 128],
                         lhsT=warm, rhs=warm, start=True, stop=True)

    # (x_pass, w_pass) pairs, dropping the doubly-residual term
    passes = [(xi, wi) for xi in range(NXP) for wi in range(NWP)
              if xi + wi <= max(NXP, NWP) - 1][:NPP]

    def qkv_fillers(j, xch):
        cs = slice(j * CH, (j + 1) * CH)
        qch = qpool.tile([128, 4, CH], BF, name=f"qch{j}", tag="qch")
        fillers = []
        dense = j == 0  # attention not running yet: borrow the sc slots

        def proj(w, dst, mb, nmb):
            def f():
                tag = "sc" if dense else "fill"
                ps = pp.tile([128, nmb * CH], F32, tag=tag, bufs=None if dense
                             else 1, name="psf")
                for t in range(nmb):
                    cols = slice((mb + t) * 128, (mb + t + 1) * 128)
                    n = 0
                    for xi, wi in passes:
                        for kp in range(KC // 2):
                            nc.tensor.matmul(
                                ps[:, t * CH:(t + 1) * CH],
                                lhsT=w[wi][:, 2 * kp:2 * kp + 2, cols],
                                rhs=xch[xi][:, 2 * kp:2 * kp + 2, :],
                                start=(n == 0),
                                stop=(n == NPP * (KC // 2) - 1),
                                perf_mode=DRM)
                            n += 1
                nc.vector.tensor_copy(dst, ps)
            return f

        def vproj(sb, nsb):
            def f():
                tag = "sc" if dense else "fill"
                ps = pp.tile([128, nsb * CH], F32, tag=tag, bufs=None if dense
                             else 1, name="psf")
                for t in range(nsb):
                    rows = slice((sb + t) * 128, (sb + t + 1) * 128)
                    n = 0
                    for xi, wi in passes:
                        for kp in range(KC // 2):
                            nc.tensor.matmul(
                                ps[:, t * CH:(t + 1) * CH],
                                lhsT=xch[xi][:, 2 * kp:2 * kp + 2, rows],
                                rhs=wv[wi][:, 2 * kp:2 * kp + 2, :],
                                start=(n == 0),
                                stop=(n == NPP * (KC // 2) - 1),
                                perf_mode=DRM)
                            n += 1
                sblk = j * 4 + sb
                psv = ps.rearrange("p (t h d) -> p t h d", t=nsb, h=HPC)
                nc.vector.tensor_copy(v1[:, sblk:sblk + nsb, :, 0:DK], psv)
                if PV_FP8:
                    nc.vector.tensor_sub(
                        v2[:, sblk:sblk + nsb, :, 0:DK], psv,
                        v1[:, sblk:sblk + nsb, :, 0:DK])
            return f

        nm = 2 if dense else 1
        for mb in range(0, 4, nm):
            fillers.append(proj(wq, qch[:, mb:mb + nm, :], mb, nm))
        kv = []  # (deadline (h, g) in chunk j's own attention loop, fn)
        for mb in range(0, 4, nm):
            # kT m-block mb is first read by head 2*mb at its step g=2j
            kv.append(((2 * mb, 2 * j - 1),
                       proj(wk, kT[:, mb:mb + nm, cs], mb, nm)))
        for sb in range(0, 4, nm):
            # v s-block 4j+sb is first read by the pv pair emitted at
            # step g = 2j + sb//2 + 2 of head 0
            kv.append(((0, 2 * j + sb // 2 + 1), vproj(sb, nm)))
        return qch, fillers, kv

    def o_fillers(j, aq):
        # chunk j's attn [q, hd] tiles: transpose back to [hd, q] on PE,
        # then the O-projection from the attnT layout.
        ach = apool.tile([128, 4, CH], BF, name=f"ach{j}", tag="ach")
        out_fns = []

        def tblk(sb):
            def f():
                tp = pp.tile([128, 4, 128], BF, tag="tp", bufs=1, name="tp")
                for hp in range(4):
                    nc.tensor.matmul(
                        tp[:, hp, :], lhsT=aq[:, sb, hp, :], rhs=ident,
                        is_transpose=True, start=True, stop=True)
                nc.vector.tensor_copy(ach[:, :, sb * 128:(sb + 1) * 128], tp)
            return f

        def oblk(sb, n):
            def f():
                sblk = j * 4 + sb
                osb = opool.tile([128, CH], F32, name="osb", tag="osb")
                ps = pp.tile([128, CH], F32, tag="fill", bufs=1, name="psf")
                for hp in range(4):
                    nc.tensor.matmul(
                        ps, lhsT=ach[:, hp, sb * 128:(sb + 1) * 128],
                        rhs=wo[:, hp, n * CH:(n + 1) * CH],
                        start=(hp == 0), stop=(hp == 3))
                nc.vector.tensor_copy(osb, ps)
                nc.sync.dma_start(
                    out=out[sblk * 128:(sblk + 1) * 128,
                            n * CH:(n + 1) * CH], in_=osb)
            return f
        for sb in range(4):
            out_fns.append(tblk(sb))
            for n in range(2):
                out_fns.append(oblk(sb, n))
        return out_fns

    pending_norm = []

    def _norm_one(at_q, aq, h):
        # Normalize a finished head: per-q reciprocal of the denominator
        # column, then one per-partition-scalar multiply per q-subtile.
        mb, half = h // 2, h % 2
        rc4 = rpool.tile([128, 4], F32, name="rc4", tag="rc4")
        nc.vector.reciprocal(out=rc4, in_=at_q[:, :, DK:DK + 1])
        for qt in range(4):
            nc.vector.tensor_scalar_mul(
                aq[:, qt, mb, half * DK:(half + 1) * DK],
                at_q[:, qt, 0:DK], rc4[:, qt:qt + 1])

    from collections import deque
    fillers = deque()   # (None, fn) or ("next", (h, g), fn)
    carry_kv = deque()  # K/V fillers deferred into the current chunk
    carry_next = deque()
    qch, f0, kv0 = qkv_fillers(0, xch0)
    for f in f0:
        f()  # nothing to overlap with at the very start
    for _, f in kv0:
        f()

    prev = None  # aq of the chunk awaiting its O-projection
    for j in range(NCH):
        # stage next chunk's x DMAs + projection fillers, and the previous
        # chunk's O-projection, to fill PE gaps in this ACT-bound phase
        if prev is not None:
            fillers.extend((None, f) for f in o_fillers(*prev))
        if j + 1 < NCH:
            xch_n = dma_x(j + 1)
            qch_n, fs, kv_n = qkv_fillers(j + 1, xch_n)
            fillers.extend((None, f) for f in fs)
            fillers.extend((None, f) for _, f in kv_n)
        else:
            qch_n = None

        aq = apool.tile([128, 4, 4, 128], BF, name=f"aq{j}", tag="aq")
        nkb = 4 * (j + 1)
        steps = HPC * (nkb // 2)
        npop = 0
        nfill0 = len(fillers) + len(carry_kv)
        gstep = 0

        closed = set()

        def emit_pv(ent):
            at_q, h, pg, pe, is_last = ent
            i0 = 2 * pg
            pe_v = pe.rearrange("p (t c) -> p t c", t=2)
            first = i0 == 0
            if PV_FP8:
                qt0 = max(0, i0 - 4 * j)
                for qt in range(qt0, 4):
                    for vi, vt in enumerate(vs):
                        nc.tensor.matmul(
                            at_q[:, qt, 0:DK + 1],
                            lhsT=pe_v[:, :, qt * 128:(qt + 1) * 128],
                            rhs=vt[:, i0:i0 + 2, h, :],
                            start=(first and qt == qt0 and vi == 0),
                            stop=(is_last and qt == 3 and vi == len(vs) - 1),
                            perf_mode=DRM, skip_group_check=True)
            else:
                for t in range(2):
                    i = i0 + t
                    qt0 = max(0, i - 4 * j)
                    for qt in range(qt0, 4):
                        nc.tensor.matmul(
                            at_q[:, qt, 0:DK + 1],
                            lhsT=pe_v[:, t, qt * 128:(qt + 1) * 128],
                            rhs=v1[:, i, h, :],
                            start=(first and t == 0 and qt == qt0),
                            stop=(is_last and t == 1 and qt == 3),
                            skip_group_check=True)
            if is_last:
                closed.add(at_q.tensor.name)

        def flush_ready():
            # emit norms only for heads whose accumulation group is closed
            # (emission order defines read/write semantics under Tile)
            while pending_norm and pending_norm[0][0].tensor.name in closed:
                at_q, h = pending_norm.pop(0)
                _norm_one(at_q, aq, h)

        pend = []
        for h in range(HPC):
            mb, half = h // 2, h % 2
            row = slice(half * DK, (half + 1) * DK)
            at_q = pp.tile([128, 4, 128], F32, tag="at", bufs=2, name="at_q")
            for g in range(nkb // 2):
                while carry_kv and carry_kv[0][0] <= (h, g):
                    carry_kv.popleft()[1]()
                i0 = 2 * g
                # Diagonal blocks are mostly masked: columns [0, qlo) of
                # k-block i are causally dead (q < k for the whole block).
                # Scores trim per block; exp reads the pair-uniform span
                # [pair_ql:) and the masks zero-fill everything the PV
                # might read that isn't a live score.
                def _qlo(i):
                    if i < 4 * j:
                        return 0
                    return min(128 * (i - 4 * j), CH - 256)

                sc = pp.tile([128, 2 * CH], F32, tag="sc", name="sc")
                pair_ql = _qlo(i0)
                for t in range(2):
                    i = i0 + t
                    nc.tensor.matmul(
                        sc[:, t * CH + pair_ql:(t + 1) * CH],
                        lhsT=kT[row, mb, i * 128:(i + 1) * 128],
                        rhs=qch[row, mb, pair_ql:], start=True, stop=True)
                e = epool.tile([128, 2 * CH], EDT, name="e", tag="e")
                sc_v = sc.rearrange("p (t c) -> p t c", t=2)[:, :, pair_ql:]
                e_v = e.rearrange("p (t c) -> p t c", t=2)[:, :, pair_ql:]
                nc.scalar.activation(out=e_v, in_=sc_v, func=EXP,
                                     scale=EXP_SCALE)
                for t in range(2):
                    i = i0 + t
                    if i >= 4 * j:
                        # columns >= 128*(d+1) of the chunk are fully valid
                        # (q > every k in this block); columns in
                        # [pair_ql, that) hold junk exp or acausal values
                        # the PV would read -- zero-fill them all.
                        hi = min(128 * (i - 4 * j + 1), CH)
                        nc.gpsimd.affine_select(
                            out=e[:, t * CH + pair_ql:t * CH + hi],
                            in_=e[:, t * CH + pair_ql:t * CH + hi],
                            compare_op=mybir.AluOpType.is_ge,
                            fill=0.0, base=j * CH - i * 128 + pair_ql,
                            channel_multiplier=-1,
                            pattern=[[1, hi - pair_ql]])
                gstep += 1
                if int(gstep * nfill0 / ((1.0 + 0.2 * j) * steps)) >= npop + 1:
                    npop += 1
                    if carry_kv:
                        carry_kv.popleft()[1]()
                    elif fillers:
                        ent = fillers.popleft()
                        f = ent[-1]
                        if ent[0] == "next":
                            carry_next.append((ent[1], f))
                        else:
                            f()
                if len(pend) > 2:
                    emit_pv(pend.pop(0))
                flush_ready()
                pend.append((at_q, h, g, e, g == nkb // 2 - 1))
            pending_norm.append((at_q, h))
        while pend:
            emit_pv(pend.pop(0))
        flush_ready()
        assert not pending_norm
        while carry_kv:
            carry_kv.popleft()[1]()
        while fillers:
            ent = fillers.popleft()
            if ent[0] == "next":
                carry_next.append((ent[1], ent[-1]))
            else:
                ent[-1]()
        carry_kv = carry_next
        carry_next = deque()
        prev = (j, aq)
        qch = qch_n

    for f in o_fillers(*prev):
        f()


def _split_excess_waits(nc, max_waits=1):
    # This walrus build rejects instructions carrying more than a couple of
    # sem waits ("Too many sync wait commands").  Engines execute their
    # stream in order, so excess waits can be moved onto nofuse nops placed
    # immediately before the instruction on the same engine.
    ctr = 0
    for blk in nc.m.functions[0].blocks:
        insts = blk.instructions
        out = []
        changed = False
        for inst in insts:
            si = inst.sync_info
            if si is not None and si.on_wait and len(si.on_wait) > max_waits:
                waits = list(si.on_wait)
                extra, keep = waits[:-max_waits], waits[-max_waits:]
                for gi in range(0, len(extra), max_waits):
                    ctr += 1
                    out.append(mybir.InstNoOp(
                        name=f"wsplit_{ctr}",
                        engine=inst.engine,
                        bass_nofuse=True,
                        sync_info=mybir.SyncInfo(
                            on_wait=extra[gi:gi + max_waits], on_update=[]),
                    ))
                inst.sync_info = mybir.SyncInfo(
                    on_wait=keep, on_update=si.on_update)
                changed = True
            out.append(inst)
        if changed:
            insts[:] = out


_CACHE = {}


def _get_nc(split=True):
    if "nc" in _CACHE:
        return _CACHE["nc"]
    tile.TileContext._drain_and_barrier = _drain_and_barrier_split
    nc = bass.Bass("TRN2", target_bir_lowering=False, debug=False)
    xTs = [nc.dram_tensor(f"xT{i}", [D, S], F8, kind="ExternalInput").ap()
           for i in range(NXP)]
    wqTs = [nc.dram_tensor(f"wqT{i}", [D, HD], F8, kind="ExternalInput").ap()
            for i in range(NWP)]
    wkTs = [nc.dram_tensor(f"wkT{i}", [D, HD], F8, kind="ExternalInput").ap()
            for i in range(NWP)]
    wvTs = [nc.dram_tensor(f"wvT{i}", [D, HD], F8, kind="ExternalInput").ap()
            for i in range(NWP)]
    woT = nc.dram_tensor("woT", [HD, D], BF, kind="ExternalInput").ap()
    out = nc.dram_tensor("out", [S, D], F32, kind="ExternalOutput").ap()
    from contextlib import ExitStack
    with tile.TileContext(nc) as tc, ExitStack() as ctx:
        _build_kernel(ctx, tc, xTs, wqTs, wkTs, wvTs, woT, out)
    if split:
        _split_excess_waits(nc)
        _CACHE["nc"] = nc
    return nc


def make_in_maps(x, Wq, Wk, Wv, Wo):
    import ml_dtypes
    F8NP = ml_dtypes.float8_e4m3
    BFNP = ml_dtypes.bfloat16
    x = np.asarray(x, np.float32)
    Wq, Wk, Wv, Wo = (np.asarray(w, np.float32) for w in (Wq, Wk, Wv, Wo))

    def q8pair(a, n):
        a1 = a.astype(F8NP)
        if n == 1:
            return [np.ascontiguousarray(a1)]
        a2 = (a - a1.astype(np.float32)).astype(F8NP)
        return [np.ascontiguousarray(a1), np.ascontiguousarray(a2)]

    in_maps = []
    for c in range(8):
        b, hh = c // 2, c % 2
        cols = slice(hh * HD, (hh + 1) * HD)
        m = {}
        for i, a in enumerate(q8pair(x[b].T, NXP)):
            m[f"xT{i}"] = a
        for i, a in enumerate(q8pair(WSC_QK * Wq[cols, :].T, NWP)):
            m[f"wqT{i}"] = a
        for i, a in enumerate(q8pair(WSC_QK * Wk[cols, :].T, NWP)):
            m[f"wkT{i}"] = a
        for i, a in enumerate(q8pair(WSC_V * Wv[cols, :].T, NWP)):
            m[f"wvT{i}"] = a
        m["woT"] = np.ascontiguousarray(Wo[:, cols].T.astype(BFNP))
        in_maps.append(m)
    return in_maps


def kernel(x, Wq, Wk, Wv, Wo, _trace=False, _trace_kwargs=None):
    nc = _get_nc()
    in_maps = make_in_maps(x, Wq, Wk, Wv, Wo)
    res = run_bass_kernel_spmd(
        nc, in_maps, core_ids=list(range(8)), trace=_trace,
        **(_trace_kwargs or {}))
    outs = [res.results[c]["out"] for c in range(8)]
    full = np.stack([outs[2 * b] + outs[2 * b + 1] for b in range(B)])
    if _trace:
        _CACHE["last_results"] = res
    return full.astype(np.float32)
